# revision 27
# baseline (speedup 1.0000x reference)
"""Crystal segment-norm kernel for 8 Trainium2 NeuronCores.

Transposed fp16 pipeline (features on partitions, atoms on the free dim):
- Host packs whole segments (padded to G=8 atoms) into 8192-atom chunks
  (<=128 segments each) via first-fit-decreasing, deals chunks round-robin
  to 8 cores (SPMD), and ships x as xT [128 feat, atoms] fp16 so every DMA
  descriptor is a contiguous 16KB line. Host also precomputes the one-hot
  group->segment masks (at) and their transposes (atT) per chunk.
- Chunk free-dim layout is replica-major: column r*1024 + j holds the r-th
  atom of group j. Group sums of x and x^2 are a 3-round contiguous fp16
  add tree (round 1 on DVE in 2x mode, rounds 2-3 on gpsimd), and the
  per-atom K/C expansion in the apply is a stride-0 middle-dim broadcast
  that keeps the innermost AP packed (2x eligible).
- PE work per chunk: 16 transposes of group-sum tiles, 8 accumulating
  seg-sum matmuls, 16 gather matmuls (K|C rows -> group columns, directly
  in feat-partition space via lhsT=KC16).
- Software pipeline: load(c+1) prefetched; apply(c-1) emitted between the
  folds and the stats of chunk c so DVE covers the gpsimd->PE->ActE
  reduction latency instead of stalling.
- Stats (mean/var/K/C) per chunk on [128 seg, 128 feat] f32 tiles follow
  the reference algebra exactly; n==1 segments fixed on host.
"""
import numpy as np

N = 1_000_000
F = 128
S = 16_384
EPS = 1e-6
VAR_FLOOR = 1e-7
NCORES = 8
G = 8
P = 128
CHUNK_ATOMS = 8192
GC = CHUNK_ATOMS // G          # 1024 groups per chunk
JT = GC // P                   # 8 j-tiles per chunk
MAXSEG = 128                   # segment slots per chunk (trash groups -> 0)
HC = CHUNK_ATOMS // 2
QC = CHUNK_ATOMS // 4


def _plan(index):
    """Pack segments into 8192-atom bins; deal bins round-robin to cores."""
    counts = np.bincount(index, minlength=S).astype(np.int64)
    seg_start = np.concatenate([[0], np.cumsum(counts)[:-1]])
    pad = ((counts + G - 1) // G) * G
    segs = np.nonzero(counts)[0]
    order = segs[np.argsort(-pad[segs], kind="stable")]
    bins = []                       # [atoms_used, nsegs, [segs...]]
    open_bins = []
    for s in order:
        p = int(pad[s])
        placed = False
        for bi in open_bins:
            b = bins[bi]
            if b[0] + p <= CHUNK_ATOMS and b[1] < MAXSEG:
                b[0] += p
                b[1] += 1
                b[2].append(s)
                placed = True
                if b[0] > CHUNK_ATOMS - G or b[1] >= MAXSEG:
                    open_bins.remove(bi)
                break
        if not placed:
            bins.append([p, 1, [s]])
            open_bins.append(len(bins) - 1)
    cores = [[] for _ in range(NCORES)]
    for i, b in enumerate(bins):
        cores[i % NCORES].append(b[2])
    nchunks = max(len(c) for c in cores)
    return cores, counts, seg_start, pad, nchunks


def _core_arrays(chunks, counts, seg_start, pad, nchunks, x16, eye16):
    nat = nchunks * CHUNK_ATOMS
    gseg = np.zeros((nchunks, GC), dtype=np.int64)     # trash groups -> slot 0
    rn = np.ones((nchunks, P), dtype=np.float32)
    rn1 = np.ones((nchunks, P), dtype=np.float32)

    seg_n, seg_src, seg_ci, seg_j0 = [], [], [], []
    n1_src = []
    for ci, segs in enumerate(chunks):
        j0 = 0
        for l, s in enumerate(segs):
            n = int(counts[s])
            ng = int(pad[s]) // G
            seg_n.append(n)
            seg_src.append(int(seg_start[s]))
            seg_ci.append(ci)
            seg_j0.append(j0)
            gseg[ci, j0:j0 + ng] = l
            rn[ci, l] = 1.0 / n
            rn1[ci, l] = 1.0 / (n - 1) if n > 1 else 1.0
            if n == 1:
                n1_src.append(int(seg_start[s]))
            j0 += ng

    seg_n = np.array(seg_n, dtype=np.int64)
    tot = int(seg_n.sum())
    starts = np.concatenate([[0], np.cumsum(seg_n)[:-1]])
    local = np.arange(tot, dtype=np.int64) - np.repeat(starts, seg_n)
    src_rows = np.repeat(np.array(seg_src, dtype=np.int64), seg_n) + local
    r = local % G
    q = local // G
    dst_cols = (np.repeat(np.array(seg_ci, dtype=np.int64), seg_n) * CHUNK_ATOMS
                + r * GC + np.repeat(np.array(seg_j0, dtype=np.int64), seg_n) + q)

    xrows = np.zeros((nat, F), dtype=np.float16)
    xrows[dst_cols] = x16[src_rows]
    xT = np.ascontiguousarray(xrows.T)

    # one-hot masks per chunk: at [128 j, 128 s] tiles, atT transposed
    atm = np.empty((P, nchunks * 2 * GC), dtype=np.float16)
    for c in range(nchunks):
        base = c * 2 * GC
        for t in range(JT):
            sl = gseg[c, t * P:(t + 1) * P]
            at_tile = eye16[sl]                     # [128 j, 128 s]
            atm[:, base + t * P:base + (t + 1) * P] = at_tile
            atm[:, base + GC + t * P:base + GC + (t + 1) * P] = at_tile.T
    gseg_dev = np.ascontiguousarray(
        gseg.astype(np.float32).reshape(nchunks, JT, P)
        .transpose(2, 0, 1).reshape(P, nchunks * JT))
    rn_dev = np.ascontiguousarray(rn.T)
    rn1_dev = np.ascontiguousarray(rn1.T)
    rnsq_dev = np.sqrt(rn_dev)
    rn1b_dev = (EPS * rn1_dev - VAR_FLOOR).astype(np.float32)
    return {
        "xT": xT,
        "atm": atm,
        "gseg": gseg_dev,
        "rn": rn_dev,
        "rn1": rn1_dev,
        "rnsq": rnsq_dev,
        "rn1b": rn1b_dev,
        "src_rows": src_rows,
        "dst_cols": dst_cols,
        "n1_src": np.array(n1_src, dtype=np.int64),
    }


def _consts(weight, bias):
    return {
        "wb": np.tile(np.asarray(weight, dtype=np.float32), (P, 1)),
        "bb": np.tile(np.asarray(bias, dtype=np.float32), (P, 1)),
        "id16": np.eye(P, dtype=np.float16),
    }


def _build(nchunks):
    import concourse.tile as tile
    from concourse import bacc, mybir

    F32 = mybir.dt.float32
    F16 = mybir.dt.float16
    AF = mybir.ActivationFunctionType
    OP = mybir.AluOpType

    nat = nchunks * CHUNK_ATOMS
    CA = CHUNK_ATOMS
    nc = bacc.Bacc("TRN2", target_bir_lowering=False, debug=False,
                   num_devices=NCORES)
    xT_d = nc.dram_tensor("xT", [P, nat], F16, kind="ExternalInput")
    outT_d = nc.dram_tensor("outT", [P, nat], F16, kind="ExternalOutput")
    atm_d = nc.dram_tensor("atm", [P, nchunks * 2 * GC], F16,
                           kind="ExternalInput")
    rn_d = nc.dram_tensor("rn", [P, nchunks], F32, kind="ExternalInput")
    rn1_d = nc.dram_tensor("rn1", [P, nchunks], F32, kind="ExternalInput")
    rnsq_d = nc.dram_tensor("rnsq", [P, nchunks], F32, kind="ExternalInput")
    rn1b_d = nc.dram_tensor("rn1b", [P, nchunks], F32, kind="ExternalInput")
    id_d = nc.dram_tensor("id16", [P, P], F16, kind="ExternalInput")
    wb_d = nc.dram_tensor("wb", [P, P], F32, kind="ExternalInput")
    bb_d = nc.dram_tensor("bb", [P, P], F32, kind="ExternalInput")

    with tile.TileContext(nc) as tc:
        with (
            tc.tile_pool(name="consts", bufs=1) as cpool,
            tc.tile_pool(name="xc", bufs=4) as xcp,
            tc.tile_pool(name="xsq", bufs=2) as xsqp,
            tc.tile_pool(name="s1", bufs=2) as s1p,
            tc.tile_pool(name="s2", bufs=2) as s2p,
            tc.tile_pool(name="g", bufs=3) as gp,
            tc.tile_pool(name="atm", bufs=3) as atp,
            tc.tile_pool(name="grp", bufs=2) as grpp,
            tc.tile_pool(name="kc", bufs=2) as kcp,
            tc.tile_pool(name="kcg", bufs=3) as kcgp,
            tc.tile_pool(name="st", bufs=2) as stp,
            tc.tile_pool(name="ps_a", bufs=2, space="PSUM") as ps_a,
            tc.tile_pool(name="ps_seg", bufs=2, space="PSUM") as ps_seg,
            tc.tile_pool(name="ps_kcgk", bufs=1, space="PSUM") as ps_kcgk,
            tc.tile_pool(name="ps_kcgc", bufs=1, space="PSUM") as ps_kcgc,
        ):
            id_t = cpool.tile([P, P], F16)
            nc.gpsimd.dma_start(out=id_t[:], in_=id_d.ap()[:, :])
            wb_t = cpool.tile([P, P], F32)
            nc.gpsimd.dma_start(out=wb_t[:], in_=wb_d.ap()[:, :])
            bb_t = cpool.tile([P, P], F32)
            nc.gpsimd.dma_start(out=bb_t[:], in_=bb_d.ap()[:, :])
            rn_t = cpool.tile([P, nchunks], F32)
            nc.gpsimd.dma_start(out=rn_t[:], in_=rn_d.ap()[:, :])
            rn1_t = cpool.tile([P, nchunks], F32)
            nc.gpsimd.dma_start(out=rn1_t[:], in_=rn1_d.ap()[:, :])
            rnsq_t = cpool.tile([P, nchunks], F32)
            nc.gpsimd.dma_start(out=rnsq_t[:], in_=rnsq_d.ap()[:, :])
            rn1b_t = cpool.tile([P, nchunks], F32)
            nc.gpsimd.dma_start(out=rn1b_t[:], in_=rn1b_d.ap()[:, :])
            floor_t = cpool.tile([P, 1], F32)
            nc.gpsimd.memset(floor_t[:], float(VAR_FLOOR))

            def emit_load(c):
                xc = xcp.tile([P, CA], F16, tag="xc")
                nc.sync.dma_start(out=xc[:],
                                  in_=xT_d.ap()[:, c * CA:(c + 1) * CA])
                atm = atp.tile([P, 2 * GC], F16, tag="atm")
                nc.sync.dma_start(
                    out=atm[:],
                    in_=atm_d.ap()[:, c * 2 * GC:(c + 1) * 2 * GC])
                return [c, xc, atm, None, None]

            def emit_front(ctx):
                """Squares + fold trees for chunk c."""
                c, xc, atm = ctx[0], ctx[1], ctx[2]
                xsq = xsqp.tile([P, CA], F16, tag="xsq")
                for h in range(2):
                    nc.scalar.activation(
                        out=xsq[:, h * HC:(h + 1) * HC],
                        in_=xc[:, h * HC:(h + 1) * HC], func=AF.Square,
                    )
                g = gp.tile([P, 2 * GC], F16, tag="g")  # [gx | gq]
                s1 = s1p.tile([P, HC], F16, tag="s1")
                nc.vector.tensor_tensor(
                    out=s1[:], in0=xc[:, 0:HC], in1=xc[:, HC:2 * HC], op=OP.add)
                s1q = s1p.tile([P, HC], F16, tag="s1")
                nc.vector.tensor_tensor(
                    out=s1q[:], in0=xsq[:, 0:HC], in1=xsq[:, HC:2 * HC],
                    op=OP.add)
                s2 = s2p.tile([P, QC], F16, tag="s2")
                nc.gpsimd.tensor_tensor(
                    out=s2[:], in0=s1[:, 0:QC], in1=s1[:, QC:2 * QC], op=OP.add)
                nc.gpsimd.tensor_tensor(
                    out=g[:, 0:GC], in0=s2[:, 0:GC], in1=s2[:, GC:2 * GC],
                    op=OP.add)
                s2q = s2p.tile([P, QC], F16, tag="s2")
                nc.gpsimd.tensor_tensor(
                    out=s2q[:], in0=s1q[:, 0:QC], in1=s1q[:, QC:2 * QC],
                    op=OP.add)
                nc.gpsimd.tensor_tensor(
                    out=g[:, GC:2 * GC], in0=s2q[:, 0:GC], in1=s2q[:, GC:2 * GC],
                    op=OP.add)
                ctx[3] = g

            def emit_reduce(ctx):
                """Group-sum transposes + accumulating seg sums (PE/ActE)."""
                c, xc, atm, g = ctx[0], ctx[1], ctx[2], ctx[3]
                grp = grpp.tile([P, 2 * GC], F16, tag="grp")
                for t in range(JT):
                    tps = ps_a.tile([P, 2 * P], F16, space="PSUM", tag="psa")
                    nc.tensor.transpose(out=tps[:, 0:P],
                                        in_=g[:, t * P:(t + 1) * P],
                                        identity=id_t[:])
                    nc.tensor.transpose(out=tps[:, P:2 * P],
                                        in_=g[:, GC + t * P:GC + (t + 1) * P],
                                        identity=id_t[:])
                    nc.scalar.activation(
                        out=grp[:, t * 2 * P:(t + 1) * 2 * P], in_=tps[:],
                        func=AF.Copy)
                seg_ps = ps_seg.tile([P, 2 * P], F32, space="PSUM", tag="seg")
                for t in range(JT):
                    nc.tensor.matmul(
                        out=seg_ps[:], lhsT=atm[:, t * P:(t + 1) * P],
                        rhs=grp[:, t * 2 * P:(t + 1) * 2 * P],
                        start=(t == 0), stop=(t == JT - 1))
                return seg_ps

            def emit_stats(ctx, seg_ps):
                """Per-chunk mean/var -> K|C fp16, then K/C gather (PE).

                ActE-heavy: mean/sum^2/var-clamp ride activation's per-
                partition scale/bias. u = relu(d*rn1 + (eps*rn1 - floor)),
                std = sqrt(u + floor) == sqrt(max(var, floor)).
                """
                c, xc, atm = ctx[0], ctx[1], ctx[2]
                kc16 = kcp.tile([P, 2 * P], F16, tag="kc16")
                mean_t = stp.tile([P, P], F32, tag="mean")
                nc.scalar.activation(out=mean_t[:], in_=seg_ps[:, 0:P],
                                     func=AF.Copy, scale=rn_t[:, c:c + 1])
                m2_t = stp.tile([P, P], F32, tag="m2")
                nc.scalar.activation(out=m2_t[:], in_=seg_ps[:, 0:P],
                                     func=AF.Square, scale=rnsq_t[:, c:c + 1])
                t2 = stp.tile([P, P], F32, tag="t2")
                nc.vector.tensor_tensor(
                    out=t2[:], in0=seg_ps[:, P:2 * P], in1=m2_t[:],
                    op=OP.subtract)
                u_t = stp.tile([P, P], F32, tag="u")
                nc.scalar.activation(out=u_t[:], in_=t2[:], func=AF.Relu,
                                     scale=rn1_t[:, c:c + 1],
                                     bias=rn1b_t[:, c:c + 1])
                std_t = stp.tile([P, P], F32, tag="std")
                nc.scalar.activation(out=std_t[:], in_=u_t[:], func=AF.Sqrt,
                                     bias=floor_t[:])
                nc.scalar.activation(out=std_t[:], in_=std_t[:], func=AF.Copy,
                                     bias=float(EPS))
                rstd_t = stp.tile([P, P], F32, tag="rstd")
                nc.vector.reciprocal(out=rstd_t[:], in_=std_t[:])
                nc.vector.tensor_tensor(
                    out=kc16[:, 0:P], in0=rstd_t[:], in1=wb_t[:], op=OP.mult)
                mk_t = stp.tile([P, P], F32, tag="mk")
                nc.gpsimd.tensor_tensor(
                    out=mk_t[:], in0=mean_t[:], in1=kc16[:, 0:P], op=OP.mult)
                nc.gpsimd.tensor_tensor(
                    out=kc16[:, P:2 * P], in0=bb_t[:], in1=mk_t[:],
                    op=OP.subtract)
                # gather K/C rows into group columns, feat-partition space
                kcKC = kcgp.tile([P, 2 * GC], F16, tag="kcKC")
                kcgK_ps = ps_kcgk.tile([P, GC], F32, space="PSUM", tag="kcgk")
                for t in range(JT):
                    nc.tensor.matmul(
                        out=kcgK_ps[:, t * P:(t + 1) * P], lhsT=kc16[:, 0:P],
                        rhs=atm[:, GC + t * P:GC + (t + 1) * P],
                        start=True, stop=True)
                nc.scalar.activation(out=kcKC[:, 0:GC], in_=kcgK_ps[:],
                                     func=AF.Copy)
                kcgC_ps = ps_kcgc.tile([P, GC], F32, space="PSUM", tag="kcgc")
                for t in range(JT):
                    nc.tensor.matmul(
                        out=kcgC_ps[:, t * P:(t + 1) * P],
                        lhsT=kc16[:, P:2 * P],
                        rhs=atm[:, GC + t * P:GC + (t + 1) * P],
                        start=True, stop=True)
                nc.scalar.activation(out=kcKC[:, GC:2 * GC], in_=kcgC_ps[:],
                                     func=AF.Copy)
                ctx[4] = kcKC

            def emit_apply(ctx):
                # in-place on xc (saves an output pool); halves so the first
                # store overlaps the second half's math
                c, xc, kcKC = ctx[0], ctx[1], ctx[4]
                kb = kcKC[:, 0:GC].rearrange(
                    "p (one j) -> p one j", one=1).broadcast_to([P, G // 2, GC])
                cb = kcKC[:, GC:2 * GC].rearrange(
                    "p (one j) -> p one j", one=1).broadcast_to([P, G // 2, GC])
                for h in range(2):
                    sl = slice(h * HC, (h + 1) * HC)
                    x3 = xc[:, sl].rearrange("p (r j) -> p r j", r=G // 2)
                    nc.vector.tensor_tensor(out=x3, in0=x3, in1=kb, op=OP.mult)
                    nc.vector.tensor_tensor(out=x3, in0=x3, in1=cb, op=OP.add)
                    nc.sync.dma_start(
                        out=outT_d.ap()[:, c * CA + h * HC:c * CA + (h + 1) * HC],
                        in_=xc[:, sl])

            # 4-stage software pipeline: load@i, front@i-1, reduce+stats@i-2,
            # apply+store@i-3 — each engine's queue starts every iteration
            # with work whose inputs finished in earlier iterations.
            ctxs = {}
            for i in range(nchunks + 3):
                if i < nchunks:
                    ctxs[i] = emit_load(i)
                if 1 <= i < nchunks + 1:
                    emit_front(ctxs[i - 1])
                if 2 <= i < nchunks + 2:
                    seg_ps = emit_reduce(ctxs[i - 2])
                    emit_stats(ctxs[i - 2], seg_ps)
                if 3 <= i:
                    emit_apply(ctxs[i - 3])
                    del ctxs[i - 3]

    nc.compile()
    return nc


_BUILD_CACHE = {}


def kernel(target_fea, index, weight, bias):
    from concourse.bass_utils import run_bass_kernel_spmd

    x16 = np.asarray(target_fea, dtype=np.float16)
    idx = np.asarray(index, dtype=np.int64)
    cores, counts, seg_start, pad, nchunks = _plan(idx)
    consts = _consts(weight, bias)
    eye16 = np.eye(P, dtype=np.float16)

    core_arrays = [
        _core_arrays(chunks, counts, seg_start, pad, nchunks, x16, eye16)
        for chunks in cores
    ]
    in_maps = []
    for ca in core_arrays:
        m = {"xT": ca["xT"], "atm": ca["atm"], "rn": ca["rn"],
             "rn1": ca["rn1"], "rnsq": ca["rnsq"], "rn1b": ca["rn1b"]}
        m.update(consts)
        in_maps.append(m)

    if nchunks not in _BUILD_CACHE:
        _BUILD_CACHE[nchunks] = _build(nchunks)
    nc = _BUILD_CACHE[nchunks]

    res = run_bass_kernel_spmd(nc, in_maps, core_ids=list(range(NCORES)))

    out = np.empty((N, F), dtype=np.float32)
    bias_np = np.asarray(bias, dtype=np.float32)
    for c in range(NCORES):
        ca = core_arrays[c]
        oT = res.results[c]["outT"]
        orows = np.ascontiguousarray(oT.T)
        out[ca["src_rows"]] = orows[ca["dst_cols"]]
        if len(ca["n1_src"]):
            out[ca["n1_src"]] = bias_np
    return out


# revision 29
# speedup vs baseline: 1.0307x; 1.0307x over previous
"""Crystal segment-norm kernel for 8 Trainium2 NeuronCores.

Transposed fp16 pipeline (features on partitions, atoms on the free dim):
- Host packs whole segments (padded to G=8 atoms) into 8192-atom chunks
  (<=128 segments each) via first-fit-decreasing, deals chunks round-robin
  to 8 cores (SPMD), and ships x as xT [128 feat, atoms] fp16 so every DMA
  descriptor is a contiguous 16KB line. Host also precomputes the one-hot
  group->segment masks (at) and their transposes (atT) per chunk.
- Chunk free-dim layout is replica-major: column r*1024 + j holds the r-th
  atom of group j. Group sums of x and x^2 are a 3-round contiguous fp16
  add tree (round 1 on DVE in 2x mode, rounds 2-3 on gpsimd), and the
  per-atom K/C expansion in the apply is a stride-0 middle-dim broadcast
  that keeps the innermost AP packed (2x eligible).
- PE work per chunk: 16 transposes of group-sum tiles, 8 accumulating
  seg-sum matmuls, 16 gather matmuls (K|C rows -> group columns, directly
  in feat-partition space via lhsT=KC16).
- Software pipeline: load(c+1) prefetched; apply(c-1) emitted between the
  folds and the stats of chunk c so DVE covers the gpsimd->PE->ActE
  reduction latency instead of stalling.
- Stats (mean/var/K/C) per chunk on [128 seg, 128 feat] f32 tiles follow
  the reference algebra exactly; n==1 segments fixed on host.
"""
import numpy as np

N = 1_000_000
F = 128
S = 16_384
EPS = 1e-6
VAR_FLOOR = 1e-7
NCORES = 8
G = 8
P = 128
CHUNK_ATOMS = 8192
GC = CHUNK_ATOMS // G          # 1024 groups per chunk
JT = GC // P                   # 8 j-tiles per chunk
MAXSEG = 128                   # segment slots per chunk (trash groups -> 0)
HC = CHUNK_ATOMS // 2
QC = CHUNK_ATOMS // 4


def _plan(index):
    """Pack segments into 8192-atom bins; deal bins round-robin to cores."""
    counts = np.bincount(index, minlength=S).astype(np.int64)
    seg_start = np.concatenate([[0], np.cumsum(counts)[:-1]])
    pad = ((counts + G - 1) // G) * G
    segs = np.nonzero(counts)[0]
    order = segs[np.argsort(-pad[segs], kind="stable")]
    bins = []                       # [atoms_used, nsegs, [segs...]]
    open_bins = []
    for s in order:
        p = int(pad[s])
        placed = False
        for bi in open_bins:
            b = bins[bi]
            if b[0] + p <= CHUNK_ATOMS and b[1] < MAXSEG:
                b[0] += p
                b[1] += 1
                b[2].append(s)
                placed = True
                if b[0] > CHUNK_ATOMS - G or b[1] >= MAXSEG:
                    open_bins.remove(bi)
                break
        if not placed:
            bins.append([p, 1, [s]])
            open_bins.append(len(bins) - 1)
    cores = [[] for _ in range(NCORES)]
    for i, b in enumerate(bins):
        cores[i % NCORES].append(b[2])
    nchunks = max(len(c) for c in cores)
    return cores, counts, seg_start, pad, nchunks


def _core_arrays(chunks, counts, seg_start, pad, nchunks, x16, eye16):
    nat = nchunks * CHUNK_ATOMS
    gseg = np.zeros((nchunks, GC), dtype=np.int64)     # trash groups -> slot 0
    rn = np.ones((nchunks, P), dtype=np.float32)
    rn1 = np.ones((nchunks, P), dtype=np.float32)

    seg_n, seg_src, seg_ci, seg_j0 = [], [], [], []
    n1_src = []
    for ci, segs in enumerate(chunks):
        j0 = 0
        for l, s in enumerate(segs):
            n = int(counts[s])
            ng = int(pad[s]) // G
            seg_n.append(n)
            seg_src.append(int(seg_start[s]))
            seg_ci.append(ci)
            seg_j0.append(j0)
            gseg[ci, j0:j0 + ng] = l
            rn[ci, l] = 1.0 / n
            rn1[ci, l] = 1.0 / (n - 1) if n > 1 else 1.0
            if n == 1:
                n1_src.append(int(seg_start[s]))
            j0 += ng

    seg_n = np.array(seg_n, dtype=np.int64)
    tot = int(seg_n.sum())
    starts = np.concatenate([[0], np.cumsum(seg_n)[:-1]])
    local = np.arange(tot, dtype=np.int64) - np.repeat(starts, seg_n)
    src_rows = np.repeat(np.array(seg_src, dtype=np.int64), seg_n) + local
    r = local % G
    q = local // G
    dst_cols = (np.repeat(np.array(seg_ci, dtype=np.int64), seg_n) * CHUNK_ATOMS
                + r * GC + np.repeat(np.array(seg_j0, dtype=np.int64), seg_n) + q)

    xrows = np.zeros((nat, F), dtype=np.float16)
    xrows[dst_cols] = x16[src_rows]
    xT = np.ascontiguousarray(xrows.T)

    # one-hot masks per chunk: at [128 j, 128 s] tiles, atT transposed
    atm = np.empty((P, nchunks * 2 * GC), dtype=np.float16)
    for c in range(nchunks):
        base = c * 2 * GC
        for t in range(JT):
            sl = gseg[c, t * P:(t + 1) * P]
            at_tile = eye16[sl]                     # [128 j, 128 s]
            atm[:, base + t * P:base + (t + 1) * P] = at_tile
            atm[:, base + GC + t * P:base + GC + (t + 1) * P] = at_tile.T
    gseg_dev = np.ascontiguousarray(
        gseg.astype(np.float32).reshape(nchunks, JT, P)
        .transpose(2, 0, 1).reshape(P, nchunks * JT))
    rn_dev = np.ascontiguousarray(rn.T)
    rn1_dev = np.ascontiguousarray(rn1.T)
    rnsq_dev = np.sqrt(rn_dev)
    rn1b_dev = (EPS * rn1_dev - VAR_FLOOR).astype(np.float32)
    return {
        "xT": xT,
        "atm": atm,
        "gseg": gseg_dev,
        "rn": rn_dev,
        "rn1": rn1_dev,
        "rnsq": rnsq_dev,
        "rn1b": rn1b_dev,
        "src_rows": src_rows,
        "dst_cols": dst_cols,
        "n1_src": np.array(n1_src, dtype=np.int64),
    }


def _consts(weight, bias):
    return {
        "wb": np.tile(np.asarray(weight, dtype=np.float32), (P, 1)),
        "bb": np.tile(np.asarray(bias, dtype=np.float32), (P, 1)),
        "id16": np.eye(P, dtype=np.float16),
    }


def _build(nchunks):
    import concourse.tile as tile
    from concourse import bacc, mybir

    F32 = mybir.dt.float32
    F16 = mybir.dt.float16
    AF = mybir.ActivationFunctionType
    OP = mybir.AluOpType

    nat = nchunks * CHUNK_ATOMS
    CA = CHUNK_ATOMS
    nc = bacc.Bacc("TRN2", target_bir_lowering=False, debug=False,
                   num_devices=NCORES)
    xT_d = nc.dram_tensor("xT", [P, nat], F16, kind="ExternalInput")
    outT_d = nc.dram_tensor("outT", [P, nat], F16, kind="ExternalOutput")
    atm_d = nc.dram_tensor("atm", [P, nchunks * 2 * GC], F16,
                           kind="ExternalInput")
    rn_d = nc.dram_tensor("rn", [P, nchunks], F32, kind="ExternalInput")
    rn1_d = nc.dram_tensor("rn1", [P, nchunks], F32, kind="ExternalInput")
    rnsq_d = nc.dram_tensor("rnsq", [P, nchunks], F32, kind="ExternalInput")
    rn1b_d = nc.dram_tensor("rn1b", [P, nchunks], F32, kind="ExternalInput")
    id_d = nc.dram_tensor("id16", [P, P], F16, kind="ExternalInput")
    wb_d = nc.dram_tensor("wb", [P, P], F32, kind="ExternalInput")
    bb_d = nc.dram_tensor("bb", [P, P], F32, kind="ExternalInput")

    with tile.TileContext(nc) as tc:
        with (
            tc.tile_pool(name="consts", bufs=1) as cpool,
            tc.tile_pool(name="xc", bufs=4) as xcp,
            tc.tile_pool(name="xsq", bufs=2) as xsqp,
            tc.tile_pool(name="s1", bufs=2) as s1p,
            tc.tile_pool(name="s2", bufs=2) as s2p,
            tc.tile_pool(name="g", bufs=3) as gp,
            tc.tile_pool(name="atm", bufs=3) as atp,
            tc.tile_pool(name="grp", bufs=2) as grpp,
            tc.tile_pool(name="kc", bufs=2) as kcp,
            tc.tile_pool(name="kcg", bufs=3) as kcgp,
            tc.tile_pool(name="st", bufs=2) as stp,
            tc.tile_pool(name="ps_a", bufs=2, space="PSUM") as ps_a,
            tc.tile_pool(name="ps_seg", bufs=2, space="PSUM") as ps_seg,
            tc.tile_pool(name="ps_kcgk", bufs=1, space="PSUM") as ps_kcgk,
            tc.tile_pool(name="ps_kcgc", bufs=1, space="PSUM") as ps_kcgc,
        ):
            id_t = cpool.tile([P, P], F16)
            nc.gpsimd.dma_start(out=id_t[:], in_=id_d.ap()[:, :])
            wb_t = cpool.tile([P, P], F32)
            nc.gpsimd.dma_start(out=wb_t[:], in_=wb_d.ap()[:, :])
            bb_t = cpool.tile([P, P], F32)
            nc.gpsimd.dma_start(out=bb_t[:], in_=bb_d.ap()[:, :])
            rn_t = cpool.tile([P, nchunks], F32)
            nc.gpsimd.dma_start(out=rn_t[:], in_=rn_d.ap()[:, :])
            rn1_t = cpool.tile([P, nchunks], F32)
            nc.gpsimd.dma_start(out=rn1_t[:], in_=rn1_d.ap()[:, :])
            rnsq_t = cpool.tile([P, nchunks], F32)
            nc.gpsimd.dma_start(out=rnsq_t[:], in_=rnsq_d.ap()[:, :])
            rn1b_t = cpool.tile([P, nchunks], F32)
            nc.gpsimd.dma_start(out=rn1b_t[:], in_=rn1b_d.ap()[:, :])
            floor_t = cpool.tile([P, 1], F32)
            nc.gpsimd.memset(floor_t[:], float(VAR_FLOOR))

            def emit_load(c):
                xc = xcp.tile([P, CA], F16, tag="xc")
                nc.sync.dma_start(out=xc[:],
                                  in_=xT_d.ap()[:, c * CA:(c + 1) * CA])
                atm = atp.tile([P, 2 * GC], F16, tag="atm")
                nc.sync.dma_start(
                    out=atm[:],
                    in_=atm_d.ap()[:, c * 2 * GC:(c + 1) * 2 * GC])
                return [c, xc, atm, None, None]

            def emit_front(ctx):
                """Squares + fold trees for chunk c."""
                c, xc, atm = ctx[0], ctx[1], ctx[2]
                xsq = xsqp.tile([P, CA], F16, tag="xsq")
                for h in range(2):
                    nc.scalar.activation(
                        out=xsq[:, h * HC:(h + 1) * HC],
                        in_=xc[:, h * HC:(h + 1) * HC], func=AF.Square,
                    )
                g = gp.tile([P, 2 * GC], F16, tag="g")  # [gx | gq]
                s1 = s1p.tile([P, HC], F16, tag="s1")
                nc.vector.tensor_tensor(
                    out=s1[:], in0=xc[:, 0:HC], in1=xc[:, HC:2 * HC], op=OP.add)
                s1q = s1p.tile([P, HC], F16, tag="s1")
                nc.vector.tensor_tensor(
                    out=s1q[:], in0=xsq[:, 0:HC], in1=xsq[:, HC:2 * HC],
                    op=OP.add)
                s2 = s2p.tile([P, QC], F16, tag="s2")
                nc.gpsimd.tensor_tensor(
                    out=s2[:], in0=s1[:, 0:QC], in1=s1[:, QC:2 * QC], op=OP.add)
                nc.gpsimd.tensor_tensor(
                    out=g[:, 0:GC], in0=s2[:, 0:GC], in1=s2[:, GC:2 * GC],
                    op=OP.add)
                s2q = s2p.tile([P, QC], F16, tag="s2")
                nc.gpsimd.tensor_tensor(
                    out=s2q[:], in0=s1q[:, 0:QC], in1=s1q[:, QC:2 * QC],
                    op=OP.add)
                nc.gpsimd.tensor_tensor(
                    out=g[:, GC:2 * GC], in0=s2q[:, 0:GC], in1=s2q[:, GC:2 * GC],
                    op=OP.add)
                ctx[3] = g

            def emit_reduce(ctx):
                """Group-sum transposes + accumulating seg sums (PE/ActE)."""
                c, xc, atm, g = ctx[0], ctx[1], ctx[2], ctx[3]
                grp = grpp.tile([P, 2 * GC], F16, tag="grp")
                for t in range(JT):
                    tps = ps_a.tile([P, 2 * P], F16, space="PSUM", tag="psa")
                    nc.tensor.transpose(out=tps[:, 0:P],
                                        in_=g[:, t * P:(t + 1) * P],
                                        identity=id_t[:])
                    nc.tensor.transpose(out=tps[:, P:2 * P],
                                        in_=g[:, GC + t * P:GC + (t + 1) * P],
                                        identity=id_t[:])
                    nc.scalar.activation(
                        out=grp[:, t * 2 * P:(t + 1) * 2 * P], in_=tps[:],
                        func=AF.Copy)
                seg_ps = ps_seg.tile([P, 2 * P], F32, space="PSUM", tag="seg")
                for t in range(JT):
                    nc.tensor.matmul(
                        out=seg_ps[:], lhsT=atm[:, t * P:(t + 1) * P],
                        rhs=grp[:, t * 2 * P:(t + 1) * 2 * P],
                        start=(t == 0), stop=(t == JT - 1))
                return seg_ps

            def emit_stats(ctx, seg_ps):
                """Per-chunk mean/var -> K|C fp16, then K/C gather (PE).

                ActE-heavy: mean/sum^2/var-clamp ride activation's per-
                partition scale/bias. u = relu(d*rn1 + (eps*rn1 - floor)),
                std = sqrt(u + floor) == sqrt(max(var, floor)).
                """
                c, xc, atm = ctx[0], ctx[1], ctx[2]
                kc16 = kcp.tile([P, 2 * P], F16, tag="kc16")
                mean_t = stp.tile([P, P], F32, tag="mean")
                nc.scalar.activation(out=mean_t[:], in_=seg_ps[:, 0:P],
                                     func=AF.Copy, scale=rn_t[:, c:c + 1])
                m2_t = stp.tile([P, P], F32, tag="m2")
                nc.scalar.activation(out=m2_t[:], in_=seg_ps[:, 0:P],
                                     func=AF.Square, scale=rnsq_t[:, c:c + 1])
                t2 = stp.tile([P, P], F32, tag="t2")
                nc.vector.tensor_tensor(
                    out=t2[:], in0=seg_ps[:, P:2 * P], in1=m2_t[:],
                    op=OP.subtract)
                u_t = stp.tile([P, P], F32, tag="u")
                nc.scalar.activation(out=u_t[:], in_=t2[:], func=AF.Relu,
                                     scale=rn1_t[:, c:c + 1],
                                     bias=rn1b_t[:, c:c + 1])
                std_t = stp.tile([P, P], F32, tag="std")
                nc.scalar.activation(out=std_t[:], in_=u_t[:], func=AF.Sqrt,
                                     bias=floor_t[:])
                nc.scalar.activation(out=std_t[:], in_=std_t[:], func=AF.Copy,
                                     bias=float(EPS))
                rstd_t = stp.tile([P, P], F32, tag="rstd")
                nc.vector.reciprocal(out=rstd_t[:], in_=std_t[:])
                nc.vector.tensor_tensor(
                    out=kc16[:, 0:P], in0=rstd_t[:], in1=wb_t[:], op=OP.mult)
                mk_t = stp.tile([P, P], F32, tag="mk")
                nc.vector.tensor_tensor(
                    out=mk_t[:], in0=mean_t[:], in1=kc16[:, 0:P], op=OP.mult)
                nc.vector.tensor_tensor(
                    out=kc16[:, P:2 * P], in0=bb_t[:], in1=mk_t[:],
                    op=OP.subtract)
                # gather K/C rows into group columns, feat-partition space
                kcKC = kcgp.tile([P, 2 * GC], F16, tag="kcKC")
                kcgK_ps = ps_kcgk.tile([P, GC], F32, space="PSUM", tag="kcgk")
                for t in range(JT):
                    nc.tensor.matmul(
                        out=kcgK_ps[:, t * P:(t + 1) * P], lhsT=kc16[:, 0:P],
                        rhs=atm[:, GC + t * P:GC + (t + 1) * P],
                        start=True, stop=True)
                nc.scalar.activation(out=kcKC[:, 0:GC], in_=kcgK_ps[:],
                                     func=AF.Copy)
                kcgC_ps = ps_kcgc.tile([P, GC], F32, space="PSUM", tag="kcgc")
                for t in range(JT):
                    nc.tensor.matmul(
                        out=kcgC_ps[:, t * P:(t + 1) * P],
                        lhsT=kc16[:, P:2 * P],
                        rhs=atm[:, GC + t * P:GC + (t + 1) * P],
                        start=True, stop=True)
                nc.scalar.activation(out=kcKC[:, GC:2 * GC], in_=kcgC_ps[:],
                                     func=AF.Copy)
                ctx[4] = kcKC

            def emit_apply(ctx):
                # in-place on xc (saves an output pool); halves so the first
                # store overlaps the second half's math
                c, xc, kcKC = ctx[0], ctx[1], ctx[4]
                kb = kcKC[:, 0:GC].rearrange(
                    "p (one j) -> p one j", one=1).broadcast_to([P, G // 2, GC])
                cb = kcKC[:, GC:2 * GC].rearrange(
                    "p (one j) -> p one j", one=1).broadcast_to([P, G // 2, GC])
                for h in range(2):
                    sl = slice(h * HC, (h + 1) * HC)
                    x3 = xc[:, sl].rearrange("p (r j) -> p r j", r=G // 2)
                    nc.vector.tensor_tensor(out=x3, in0=x3, in1=kb, op=OP.mult)
                    nc.vector.tensor_tensor(out=x3, in0=x3, in1=cb, op=OP.add)
                    nc.sync.dma_start(
                        out=outT_d.ap()[:, c * CA + h * HC:c * CA + (h + 1) * HC],
                        in_=xc[:, sl])

            # 4-stage software pipeline: load@i, front@i-1, reduce+stats@i-2,
            # apply+store@i-3 — each engine's queue starts every iteration
            # with work whose inputs finished in earlier iterations.
            # apply(i-3) right after the load issue: it is the one stage whose
            # inputs are guaranteed ready, so it must sit at the head of the
            # DVE queue each iteration instead of behind the folds.
            ctxs = {}
            for i in range(nchunks + 3):
                if i < nchunks:
                    ctxs[i] = emit_load(i)
                if 3 <= i:
                    emit_apply(ctxs[i - 3])
                    del ctxs[i - 3]
                if 1 <= i < nchunks + 1:
                    emit_front(ctxs[i - 1])
                if 2 <= i < nchunks + 2:
                    seg_ps = emit_reduce(ctxs[i - 2])
                    emit_stats(ctxs[i - 2], seg_ps)

    nc.compile()
    return nc


_BUILD_CACHE = {}


def kernel(target_fea, index, weight, bias):
    from concourse.bass_utils import run_bass_kernel_spmd

    x16 = np.asarray(target_fea, dtype=np.float16)
    idx = np.asarray(index, dtype=np.int64)
    cores, counts, seg_start, pad, nchunks = _plan(idx)
    consts = _consts(weight, bias)
    eye16 = np.eye(P, dtype=np.float16)

    core_arrays = [
        _core_arrays(chunks, counts, seg_start, pad, nchunks, x16, eye16)
        for chunks in cores
    ]
    in_maps = []
    for ca in core_arrays:
        m = {"xT": ca["xT"], "atm": ca["atm"], "rn": ca["rn"],
             "rn1": ca["rn1"], "rnsq": ca["rnsq"], "rn1b": ca["rn1b"]}
        m.update(consts)
        in_maps.append(m)

    if nchunks not in _BUILD_CACHE:
        _BUILD_CACHE[nchunks] = _build(nchunks)
    nc = _BUILD_CACHE[nchunks]

    res = run_bass_kernel_spmd(nc, in_maps, core_ids=list(range(NCORES)))

    out = np.empty((N, F), dtype=np.float32)
    bias_np = np.asarray(bias, dtype=np.float32)
    for c in range(NCORES):
        ca = core_arrays[c]
        oT = res.results[c]["outT"]
        orows = np.ascontiguousarray(oT.T)
        out[ca["src_rows"]] = orows[ca["dst_cols"]]
        if len(ca["n1_src"]):
            out[ca["n1_src"]] = bias_np
    return out


# revision 33
# speedup vs baseline: 1.0374x; 1.0065x over previous
"""Crystal segment-norm kernel for 8 Trainium2 NeuronCores.

Transposed fp16 pipeline (features on partitions, atoms on the free dim):
- Host packs whole segments (padded to G=8 atoms) into 8192-atom chunks
  (<=128 segments each) via first-fit-decreasing, deals chunks round-robin
  to 8 cores (SPMD), and ships x as xT [128 feat, atoms] fp16 so every DMA
  descriptor is a contiguous 16KB line. Host also precomputes the one-hot
  group->segment masks (at) and their transposes (atT) per chunk.
- Chunk free-dim layout is replica-major: column r*1024 + j holds the r-th
  atom of group j. Group sums of x and x^2 are a 3-round contiguous fp16
  add tree (round 1 on DVE in 2x mode, rounds 2-3 on gpsimd), and the
  per-atom K/C expansion in the apply is a stride-0 middle-dim broadcast
  that keeps the innermost AP packed (2x eligible).
- PE work per chunk: 16 transposes of group-sum tiles, 8 accumulating
  seg-sum matmuls, 16 gather matmuls (K|C rows -> group columns, directly
  in feat-partition space via lhsT=KC16).
- Software pipeline: load(c+1) prefetched; apply(c-1) emitted between the
  folds and the stats of chunk c so DVE covers the gpsimd->PE->ActE
  reduction latency instead of stalling.
- Stats (mean/var/K/C) per chunk on [128 seg, 128 feat] f32 tiles follow
  the reference algebra exactly; n==1 segments fixed on host.
"""
import numpy as np

N = 1_000_000
F = 128
S = 16_384
EPS = 1e-6
VAR_FLOOR = 1e-7
NCORES = 8
G = 8
P = 128
CHUNK_ATOMS = 8192
GC = CHUNK_ATOMS // G          # 1024 groups per chunk
JT = GC // P                   # 8 j-tiles per chunk
MAXSEG = 128                   # segment slots per chunk (trash groups -> 0)
HC = CHUNK_ATOMS // 2
QC = CHUNK_ATOMS // 4


def _plan(index):
    """Pack segments into 8192-atom bins; deal bins round-robin to cores."""
    counts = np.bincount(index, minlength=S).astype(np.int64)
    seg_start = np.concatenate([[0], np.cumsum(counts)[:-1]])
    pad = ((counts + G - 1) // G) * G
    segs = np.nonzero(counts)[0]
    order = segs[np.argsort(-pad[segs], kind="stable")]
    bins = []                       # [atoms_used, nsegs, [segs...]]
    open_bins = []
    for s in order:
        p = int(pad[s])
        placed = False
        for bi in open_bins:
            b = bins[bi]
            if b[0] + p <= CHUNK_ATOMS and b[1] < MAXSEG:
                b[0] += p
                b[1] += 1
                b[2].append(s)
                placed = True
                if b[0] > CHUNK_ATOMS - G or b[1] >= MAXSEG:
                    open_bins.remove(bi)
                break
        if not placed:
            bins.append([p, 1, [s]])
            open_bins.append(len(bins) - 1)
    cores = [[] for _ in range(NCORES)]
    for i, b in enumerate(bins):
        cores[i % NCORES].append(b[2])
    nchunks = max(len(c) for c in cores)
    return cores, counts, seg_start, pad, nchunks


def _core_arrays(chunks, counts, seg_start, pad, nchunks, x16, eye16):
    nat = nchunks * CHUNK_ATOMS
    gseg = np.zeros((nchunks, GC), dtype=np.int64)     # trash groups -> slot 0
    rn = np.ones((nchunks, P), dtype=np.float32)
    rn1 = np.ones((nchunks, P), dtype=np.float32)

    seg_n, seg_src, seg_ci, seg_j0 = [], [], [], []
    n1_src = []
    for ci, segs in enumerate(chunks):
        j0 = 0
        for l, s in enumerate(segs):
            n = int(counts[s])
            ng = int(pad[s]) // G
            seg_n.append(n)
            seg_src.append(int(seg_start[s]))
            seg_ci.append(ci)
            seg_j0.append(j0)
            gseg[ci, j0:j0 + ng] = l
            rn[ci, l] = 1.0 / n
            rn1[ci, l] = 1.0 / (n - 1) if n > 1 else 1.0
            if n == 1:
                n1_src.append(int(seg_start[s]))
            j0 += ng

    seg_n = np.array(seg_n, dtype=np.int64)
    tot = int(seg_n.sum())
    starts = np.concatenate([[0], np.cumsum(seg_n)[:-1]])
    local = np.arange(tot, dtype=np.int64) - np.repeat(starts, seg_n)
    src_rows = np.repeat(np.array(seg_src, dtype=np.int64), seg_n) + local
    r = local % G
    q = local // G
    dst_cols = (np.repeat(np.array(seg_ci, dtype=np.int64), seg_n) * CHUNK_ATOMS
                + r * GC + np.repeat(np.array(seg_j0, dtype=np.int64), seg_n) + q)

    xrows = np.zeros((nat, F), dtype=np.float16)
    xrows[dst_cols] = x16[src_rows]
    xT = np.ascontiguousarray(xrows.T)

    # one-hot masks per chunk: at [128 j, 128 s] tiles, atT transposed
    atm = np.empty((P, nchunks * 2 * GC), dtype=np.float16)
    for c in range(nchunks):
        base = c * 2 * GC
        for t in range(JT):
            sl = gseg[c, t * P:(t + 1) * P]
            at_tile = eye16[sl]                     # [128 j, 128 s]
            atm[:, base + t * P:base + (t + 1) * P] = at_tile
            atm[:, base + GC + t * P:base + GC + (t + 1) * P] = at_tile.T
    gseg_dev = np.ascontiguousarray(
        gseg.astype(np.float32).reshape(nchunks, JT, P)
        .transpose(2, 0, 1).reshape(P, nchunks * JT))
    rn_dev = np.ascontiguousarray(rn.T)
    rn1_dev = np.ascontiguousarray(rn1.T)
    rnsq_dev = np.sqrt(rn_dev)
    rn1b_dev = (EPS * rn1_dev - VAR_FLOOR).astype(np.float32)
    return {
        "xT": xT,
        "atm": atm,
        "gseg": gseg_dev,
        "rn": rn_dev,
        "rn1": rn1_dev,
        "rnsq": rnsq_dev,
        "rn1b": rn1b_dev,
        "src_rows": src_rows,
        "dst_cols": dst_cols,
        "n1_src": np.array(n1_src, dtype=np.int64),
    }


def _consts(weight, bias):
    return {
        "wb": np.tile(np.asarray(weight, dtype=np.float32), (P, 1)),
        "bb": np.tile(np.asarray(bias, dtype=np.float32), (P, 1)),
        "id16": np.eye(P, dtype=np.float16),
    }


def _build(nchunks):
    import concourse.tile as tile
    from concourse import bacc, mybir

    F32 = mybir.dt.float32
    F16 = mybir.dt.float16
    AF = mybir.ActivationFunctionType
    OP = mybir.AluOpType

    nat = nchunks * CHUNK_ATOMS
    CA = CHUNK_ATOMS
    nc = bacc.Bacc("TRN2", target_bir_lowering=False, debug=False,
                   num_devices=NCORES)
    xT_d = nc.dram_tensor("xT", [P, nat], F16, kind="ExternalInput")
    outT_d = nc.dram_tensor("outT", [P, nat], F16, kind="ExternalOutput")
    atm_d = nc.dram_tensor("atm", [P, nchunks * 2 * GC], F16,
                           kind="ExternalInput")
    rn_d = nc.dram_tensor("rn", [P, nchunks], F32, kind="ExternalInput")
    rn1_d = nc.dram_tensor("rn1", [P, nchunks], F32, kind="ExternalInput")
    rnsq_d = nc.dram_tensor("rnsq", [P, nchunks], F32, kind="ExternalInput")
    rn1b_d = nc.dram_tensor("rn1b", [P, nchunks], F32, kind="ExternalInput")
    id_d = nc.dram_tensor("id16", [P, P], F16, kind="ExternalInput")
    wb_d = nc.dram_tensor("wb", [P, P], F32, kind="ExternalInput")
    bb_d = nc.dram_tensor("bb", [P, P], F32, kind="ExternalInput")

    with tile.TileContext(nc) as tc:
        with (
            tc.tile_pool(name="consts", bufs=1) as cpool,
            tc.tile_pool(name="xc", bufs=5) as xcp,
            tc.tile_pool(name="xsq", bufs=2) as xsqp,
            tc.tile_pool(name="s1", bufs=2) as s1p,
            tc.tile_pool(name="s2", bufs=2) as s2p,
            tc.tile_pool(name="g", bufs=3) as gp,
            tc.tile_pool(name="atm", bufs=3) as atp,
            tc.tile_pool(name="grp", bufs=2) as grpp,
            tc.tile_pool(name="kc", bufs=2) as kcp,
            tc.tile_pool(name="kcg", bufs=4) as kcgp,
            tc.tile_pool(name="st", bufs=2) as stp,
            tc.tile_pool(name="ps_a", bufs=2, space="PSUM") as ps_a,
            tc.tile_pool(name="ps_seg", bufs=2, space="PSUM") as ps_seg,
            tc.tile_pool(name="ps_kcgk", bufs=1, space="PSUM") as ps_kcgk,
            tc.tile_pool(name="ps_kcgc", bufs=1, space="PSUM") as ps_kcgc,
        ):
            id_t = cpool.tile([P, P], F16)
            nc.gpsimd.dma_start(out=id_t[:], in_=id_d.ap()[:, :])
            wb_t = cpool.tile([P, P], F32)
            nc.gpsimd.dma_start(out=wb_t[:], in_=wb_d.ap()[:, :])
            bb_t = cpool.tile([P, P], F32)
            nc.gpsimd.dma_start(out=bb_t[:], in_=bb_d.ap()[:, :])
            rn_t = cpool.tile([P, nchunks], F32)
            nc.gpsimd.dma_start(out=rn_t[:], in_=rn_d.ap()[:, :])
            rn1_t = cpool.tile([P, nchunks], F32)
            nc.gpsimd.dma_start(out=rn1_t[:], in_=rn1_d.ap()[:, :])
            rnsq_t = cpool.tile([P, nchunks], F32)
            nc.gpsimd.dma_start(out=rnsq_t[:], in_=rnsq_d.ap()[:, :])
            rn1b_t = cpool.tile([P, nchunks], F32)
            nc.gpsimd.dma_start(out=rn1b_t[:], in_=rn1b_d.ap()[:, :])
            floor_t = cpool.tile([P, 1], F32)
            nc.gpsimd.memset(floor_t[:], float(VAR_FLOOR))

            def emit_load(c):
                xc = xcp.tile([P, CA], F16, tag="xc")
                nc.sync.dma_start(out=xc[:],
                                  in_=xT_d.ap()[:, c * CA:(c + 1) * CA])
                atm = atp.tile([P, 2 * GC], F16, tag="atm")
                nc.sync.dma_start(
                    out=atm[:],
                    in_=atm_d.ap()[:, c * 2 * GC:(c + 1) * 2 * GC])
                return [c, xc, atm, None, None]

            def emit_front(ctx):
                """Squares + fold trees for chunk c."""
                c, xc, atm = ctx[0], ctx[1], ctx[2]
                xsq = xsqp.tile([P, CA], F16, tag="xsq")
                for h in range(2):
                    nc.scalar.activation(
                        out=xsq[:, h * HC:(h + 1) * HC],
                        in_=xc[:, h * HC:(h + 1) * HC], func=AF.Square,
                    )
                g = gp.tile([P, 2 * GC], F16, tag="g")  # [gx | gq]
                s1 = s1p.tile([P, HC], F16, tag="s1")
                nc.vector.tensor_tensor(
                    out=s1[:], in0=xc[:, 0:HC], in1=xc[:, HC:2 * HC], op=OP.add)
                s1q = s1p.tile([P, HC], F16, tag="s1")
                nc.vector.tensor_tensor(
                    out=s1q[:], in0=xsq[:, 0:HC], in1=xsq[:, HC:2 * HC],
                    op=OP.add)
                s2 = s2p.tile([P, QC], F16, tag="s2")
                nc.gpsimd.tensor_tensor(
                    out=s2[:], in0=s1[:, 0:QC], in1=s1[:, QC:2 * QC], op=OP.add)
                nc.gpsimd.tensor_tensor(
                    out=g[:, 0:GC], in0=s2[:, 0:GC], in1=s2[:, GC:2 * GC],
                    op=OP.add)
                s2q = s2p.tile([P, QC], F16, tag="s2")
                nc.gpsimd.tensor_tensor(
                    out=s2q[:], in0=s1q[:, 0:QC], in1=s1q[:, QC:2 * QC],
                    op=OP.add)
                nc.gpsimd.tensor_tensor(
                    out=g[:, GC:2 * GC], in0=s2q[:, 0:GC], in1=s2q[:, GC:2 * GC],
                    op=OP.add)
                ctx[3] = g

            def emit_reduce(ctx):
                """Group-sum transposes + accumulating seg sums (PE/ActE)."""
                c, xc, atm, g = ctx[0], ctx[1], ctx[2], ctx[3]
                grp = grpp.tile([P, 2 * GC], F16, tag="grp")
                for t in range(JT):
                    tps = ps_a.tile([P, 2 * P], F16, space="PSUM", tag="psa")
                    nc.tensor.transpose(out=tps[:, 0:P],
                                        in_=g[:, t * P:(t + 1) * P],
                                        identity=id_t[:])
                    nc.tensor.transpose(out=tps[:, P:2 * P],
                                        in_=g[:, GC + t * P:GC + (t + 1) * P],
                                        identity=id_t[:])
                    nc.scalar.activation(
                        out=grp[:, t * 2 * P:(t + 1) * 2 * P], in_=tps[:],
                        func=AF.Copy)
                seg_ps = ps_seg.tile([P, 2 * P], F32, space="PSUM", tag="seg")
                for t in range(JT):
                    nc.tensor.matmul(
                        out=seg_ps[:], lhsT=atm[:, t * P:(t + 1) * P],
                        rhs=grp[:, t * 2 * P:(t + 1) * 2 * P],
                        start=(t == 0), stop=(t == JT - 1))
                return seg_ps

            def emit_stats(ctx, seg_ps):
                """Per-chunk mean/var -> K|C fp16, then K/C gather (PE).

                ActE-heavy: mean/sum^2/var-clamp ride activation's per-
                partition scale/bias. u = relu(d*rn1 + (eps*rn1 - floor)),
                std = sqrt(u + floor) == sqrt(max(var, floor)).
                """
                c, xc, atm = ctx[0], ctx[1], ctx[2]
                kc16 = kcp.tile([P, 2 * P], F16, tag="kc16")
                mean_t = stp.tile([P, P], F32, tag="mean")
                nc.scalar.activation(out=mean_t[:], in_=seg_ps[:, 0:P],
                                     func=AF.Copy, scale=rn_t[:, c:c + 1])
                m2_t = stp.tile([P, P], F32, tag="m2")
                nc.scalar.activation(out=m2_t[:], in_=seg_ps[:, 0:P],
                                     func=AF.Square, scale=rnsq_t[:, c:c + 1])
                t2 = stp.tile([P, P], F32, tag="t2")
                nc.vector.tensor_tensor(
                    out=t2[:], in0=seg_ps[:, P:2 * P], in1=m2_t[:],
                    op=OP.subtract)
                u_t = stp.tile([P, P], F32, tag="u")
                nc.scalar.activation(out=u_t[:], in_=t2[:], func=AF.Relu,
                                     scale=rn1_t[:, c:c + 1],
                                     bias=rn1b_t[:, c:c + 1])
                std_t = stp.tile([P, P], F32, tag="std")
                nc.scalar.activation(out=std_t[:], in_=u_t[:], func=AF.Sqrt,
                                     bias=floor_t[:])
                nc.scalar.activation(out=std_t[:], in_=std_t[:], func=AF.Copy,
                                     bias=float(EPS))
                rstd_t = stp.tile([P, P], F32, tag="rstd")
                nc.vector.reciprocal(out=rstd_t[:], in_=std_t[:])
                nc.vector.tensor_tensor(
                    out=kc16[:, 0:P], in0=rstd_t[:], in1=wb_t[:], op=OP.mult)
                mk_t = stp.tile([P, P], F32, tag="mk")
                nc.vector.tensor_tensor(
                    out=mk_t[:], in0=mean_t[:], in1=kc16[:, 0:P], op=OP.mult)
                nc.vector.tensor_tensor(
                    out=kc16[:, P:2 * P], in0=bb_t[:], in1=mk_t[:],
                    op=OP.subtract)
                # gather K/C rows into group columns, feat-partition space
                kcKC = kcgp.tile([P, 2 * GC], F16, tag="kcKC")
                kcgK_ps = ps_kcgk.tile([P, GC], F32, space="PSUM", tag="kcgk")
                for t in range(JT):
                    nc.tensor.matmul(
                        out=kcgK_ps[:, t * P:(t + 1) * P], lhsT=kc16[:, 0:P],
                        rhs=atm[:, GC + t * P:GC + (t + 1) * P],
                        start=True, stop=True)
                nc.scalar.activation(out=kcKC[:, 0:GC], in_=kcgK_ps[:],
                                     func=AF.Copy)
                kcgC_ps = ps_kcgc.tile([P, GC], F32, space="PSUM", tag="kcgc")
                for t in range(JT):
                    nc.tensor.matmul(
                        out=kcgC_ps[:, t * P:(t + 1) * P],
                        lhsT=kc16[:, P:2 * P],
                        rhs=atm[:, GC + t * P:GC + (t + 1) * P],
                        start=True, stop=True)
                nc.scalar.activation(out=kcKC[:, GC:2 * GC], in_=kcgC_ps[:],
                                     func=AF.Copy)
                ctx[4] = kcKC

            def emit_apply(ctx):
                # in-place on xc; two full-chunk ops, one store
                c, xc, kcKC = ctx[0], ctx[1], ctx[4]
                kb = kcKC[:, 0:GC].rearrange(
                    "p (one j) -> p one j", one=1).broadcast_to([P, G, GC])
                cb = kcKC[:, GC:2 * GC].rearrange(
                    "p (one j) -> p one j", one=1).broadcast_to([P, G, GC])
                x3 = xc[:].rearrange("p (r j) -> p r j", r=G)
                nc.vector.tensor_tensor(out=x3, in0=x3, in1=kb, op=OP.mult)
                nc.vector.tensor_tensor(out=x3, in0=x3, in1=cb, op=OP.add)
                nc.sync.dma_start(
                    out=outT_d.ap()[:, c * CA:(c + 1) * CA], in_=xc[:])

            # 4-stage software pipeline: load@i, front@i-1, reduce+stats@i-2,
            # apply+store@i-3 — each engine's queue starts every iteration
            # with work whose inputs finished in earlier iterations.
            # apply(i-3) right after the load issue: it is the one stage whose
            # inputs are guaranteed ready, so it must sit at the head of the
            # DVE queue each iteration instead of behind the folds.
            ctxs = {}
            for i in range(nchunks + 4):
                if i < nchunks:
                    ctxs[i] = emit_load(i)
                if 4 <= i:
                    emit_apply(ctxs[i - 4])
                    del ctxs[i - 4]
                if 1 <= i < nchunks + 1:
                    emit_front(ctxs[i - 1])
                if 2 <= i < nchunks + 2:
                    seg_ps = emit_reduce(ctxs[i - 2])
                    emit_stats(ctxs[i - 2], seg_ps)

    nc.compile()
    return nc


_BUILD_CACHE = {}


def kernel(target_fea, index, weight, bias):
    from concourse.bass_utils import run_bass_kernel_spmd

    x16 = np.asarray(target_fea, dtype=np.float16)
    idx = np.asarray(index, dtype=np.int64)
    cores, counts, seg_start, pad, nchunks = _plan(idx)
    consts = _consts(weight, bias)
    eye16 = np.eye(P, dtype=np.float16)

    core_arrays = [
        _core_arrays(chunks, counts, seg_start, pad, nchunks, x16, eye16)
        for chunks in cores
    ]
    in_maps = []
    for ca in core_arrays:
        m = {"xT": ca["xT"], "atm": ca["atm"], "rn": ca["rn"],
             "rn1": ca["rn1"], "rnsq": ca["rnsq"], "rn1b": ca["rn1b"]}
        m.update(consts)
        in_maps.append(m)

    if nchunks not in _BUILD_CACHE:
        _BUILD_CACHE[nchunks] = _build(nchunks)
    nc = _BUILD_CACHE[nchunks]

    res = run_bass_kernel_spmd(nc, in_maps, core_ids=list(range(NCORES)))

    out = np.empty((N, F), dtype=np.float32)
    bias_np = np.asarray(bias, dtype=np.float32)
    for c in range(NCORES):
        ca = core_arrays[c]
        oT = res.results[c]["outT"]
        orows = np.ascontiguousarray(oT.T)
        out[ca["src_rows"]] = orows[ca["dst_cols"]]
        if len(ca["n1_src"]):
            out[ca["n1_src"]] = bias_np
    return out
